# revision 3
# baseline (speedup 1.0000x reference)
"""BlockwiseQuantLinear on 8 trn2 NeuronCores.

y = act_quant_dequant(x) @ (fp8_weight * block_scales).T
  x: [8192, 2048] f32, weight: [2048, 2048] fp8_e4m3fn (OCP), w_scale: [16, 16] f32
  out: [8192, 2048] f32

Strategy (data-parallel over tokens; hardcoded shapes):
  - Host: dequantize the static weight to bf16 (exact wrt reference up to bf16
    rounding) and pre-transpose it K-major, tiled [n_chunk][k_group] for direct
    DMA into [k_inner=128, k_sub, n] SBUF tiles. Shard x rows 8 ways.
  - Device (per core, M_sh=1024): for each 128-row tile of x, per 512-wide k
    group: blockwise act quant (amax over each (1,128) k-block -> scale;
    multiply by 1/(2*scale) and cast to TRN fp8e4, which equals the OCP e4m3fn
    quantization at half scale -- TRN's max normal is 240, so the half grid
    keeps values <= 224), dequantize to bf16, DMA-transpose to [k, m]; then one
    PSUM-accumulated bf16 GEMM over all 16 k-blocks per 512-wide n chunk
    (scales fold fully into the operands, so no per-block rescale is needed).
  - DMA queues: x loads + y stores on sync(SP), transposes on scalar(ACT)
    HWDGE, weight loads on gpsimd SWDGE -- keeps the PE-critical path short.
  - Gather: concatenate the 8 row shards.
"""

import numpy as np
import ml_dtypes

import concourse.bass as bass
import concourse.mybir as mybir
import concourse.tile as tile
from concourse import bacc
from concourse.bass_utils import run_bass_kernel_spmd

P = 128
M, K, N = 8192, 2048, 2048
NCORES = 8
M_SH = M // NCORES            # 1024 rows per core
MT = M_SH // P                # 8 m-tiles per core
KB = K // P                   # 16 k blocks
KBG = 4                       # k groups (of 4 blocks = 512 wide)
KSUB = KB // KBG              # 4 k blocks per group
KG_W = KSUB * P               # 512
NCH = 4                       # n chunks of 512
NC_W = N // NCH               # 512
EPS = 1e-12

_cache = {}


def _build():
    nc = bacc.Bacc(None, target_bir_lowering=False)

    x_in = nc.dram_tensor("x_sh", [M_SH, K], mybir.dt.float32, kind="ExternalInput")
    # [n_chunk, k_group, k_inner, k_sub, n] -- each [c, g] block contiguous
    w_in = nc.dram_tensor(
        "wT", [NCH, KBG, P, KSUB, NC_W], mybir.dt.bfloat16, kind="ExternalInput"
    )
    y_out = nc.dram_tensor("y_sh", [M_SH, N], mybir.dt.float32, kind="ExternalOutput")

    with tile.TileContext(nc) as tc:
        with (
            tc.tile_pool(name="wpool", bufs=1) as wpool,
            tc.tile_pool(name="xpool", bufs=3) as xpool,
            tc.tile_pool(name="qpool", bufs=3) as qpool,
            tc.tile_pool(name="spool", bufs=3) as spool,
            tc.tile_pool(name="ypool", bufs=2) as ypool,
            tc.tile_pool(name="ps", bufs=2, space="PSUM") as ps,
        ):
            # resident weights: 16 tiles of [128, 4, 512] bf16 (64KB/partition)
            wts = [[None] * KBG for _ in range(NCH)]
            for c in range(NCH):
                for g in range(KBG):
                    wt = wpool.tile([P, KSUB, NC_W], mybir.dt.bfloat16, name=f"w{c}_{g}")
                    nc.gpsimd.dma_start(wt[:], w_in[c, g])
                    wts[c][g] = wt

            for mi in range(MT):
                xTs = []
                for g in range(KBG):
                    xg = xpool.tile([P, KG_W], mybir.dt.float32, name=f"x{g}")
                    nc.sync.dma_start(xg[:], x_in[bass.ts(mi, P), bass.ts(g, KG_W)])
                    x3 = xg[:].rearrange("p (kb ki) -> p kb ki", kb=KSUB)

                    amax = spool.tile([P, KSUB], mybir.dt.float32, name=f"amax{g}")
                    nc.vector.tensor_reduce(
                        amax[:], x3, axis=mybir.AxisListType.X,
                        op=mybir.AluOpType.max, apply_absolute_value=True,
                    )
                    s2 = spool.tile([P, KSUB], mybir.dt.float32, name=f"s2_{g}")
                    nc.vector.tensor_scalar(
                        s2[:], amax[:], EPS, 1.0 / 224.0,
                        mybir.AluOpType.max, mybir.AluOpType.mult,
                    )
                    inv2 = spool.tile([P, KSUB], mybir.dt.float32, name=f"inv2_{g}")
                    nc.vector.reciprocal(inv2[:], s2[:])

                    t8 = qpool.tile([P, KG_W], mybir.dt.float8e4, name=f"t8_{g}")
                    t83 = t8[:].rearrange("p (kb ki) -> p kb ki", kb=KSUB)
                    nc.vector.tensor_tensor(
                        t83, x3, inv2[:, :, None].to_broadcast([P, KSUB, P]),
                        mybir.AluOpType.mult,
                    )
                    xdq = qpool.tile([P, KG_W], mybir.dt.bfloat16, name=f"xdq{g}")
                    xdq3 = xdq[:].rearrange("p (kb ki) -> p kb ki", kb=KSUB)
                    nc.vector.tensor_tensor(
                        xdq3, t83, s2[:, :, None].to_broadcast([P, KSUB, P]),
                        mybir.AluOpType.mult,
                    )

                    xT = qpool.tile([P, KSUB, P], mybir.dt.bfloat16, name=f"xT{g}")
                    nc.scalar.dma_start_transpose(xT[:], xdq[:])
                    xTs.append(xT)

                for c in range(NCH):
                    psum = ps.tile([P, NC_W], mybir.dt.float32, name=f"ps{c}")
                    for kb in range(KB):
                        g, j = divmod(kb, KSUB)
                        nc.tensor.matmul(
                            psum[:], xTs[g][:, j, :], wts[c][g][:, j, :],
                            start=(kb == 0), stop=(kb == KB - 1),
                        )
                    yc = ypool.tile([P, NC_W], mybir.dt.float32, name=f"y{c}")
                    nc.any.tensor_copy(yc[:], psum[:])
                    nc.sync.dma_start(
                        y_out[bass.ts(mi, P), bass.ts(c, NC_W)], yc[:]
                    )

    nc.compile()
    return nc


def _prep_weight(weight: np.ndarray, w_scale: np.ndarray) -> np.ndarray:
    w_f32 = weight.astype(np.float32)                     # exact
    ws_full = np.repeat(np.repeat(w_scale.astype(np.float32), P, axis=0), P, axis=1)
    w_deq = (w_f32 * ws_full).astype(ml_dtypes.bfloat16)  # [N, K]
    # w_deq.T[k, n]: k = (g*KSUB + j)*P + ki, n = c*NC_W + nn
    #   -> [c, g, ki, j, nn]
    wt = np.ascontiguousarray(
        w_deq.T.reshape(KBG, KSUB, P, NCH, NC_W).transpose(3, 0, 2, 1, 4)
    )
    return wt


def kernel(x: np.ndarray, weight: np.ndarray, w_scale: np.ndarray, _trace: bool = False):
    if "nc" not in _cache:
        _cache["nc"] = _build()
    nc = _cache["nc"]

    weight = np.asarray(weight)
    w_scale = np.asarray(w_scale, dtype=np.float32)
    wt = _prep_weight(weight, w_scale)
    x = np.ascontiguousarray(np.asarray(x), dtype=np.float32)

    in_maps = [
        {"x_sh": x[c * M_SH:(c + 1) * M_SH], "wT": wt}
        for c in range(NCORES)
    ]
    res = run_bass_kernel_spmd(
        nc, in_maps, core_ids=list(range(NCORES)),
        trace=_trace, trace_cores=list(range(NCORES)) if _trace else None,
    )
    y = np.concatenate([res.results[c]["y_sh"] for c in range(NCORES)], axis=0)
    if _trace:
        kernel.last_results = res
    return y
